# revision 1
# baseline (speedup 1.0000x reference)
"""Trainium2 Bass kernel for nn_GAT_7851200217746 (hierarchical GAT message passing).

Algorithm (aggregate-first GAT restructuring, validated vs reference at 3e-6 fp32):
  For each GAT layer application on (x_self [G,F], x_neigh [G,E,F], W, a_s, a_n):
    w_s[f,h] = sum_d W[f,h*D+d] a_s[h,d];  w_n likewise
    e_s = x_self @ w_s;  e_n = x_neigh @ w_n
    alpha = softmax_E(leaky_relu(e_s + e_n))            (no max-subtraction; logits |x|<3)
    x_agg[g,h,:] = sum_e alpha[g,e,h] x_neigh[g,e,:]    (aggregate in INPUT feature space)
    out[g, h*D:(h+1)*D] = x_agg[g,h,:] @ W[:, h*D:(h+1)*D]
  This avoids materializing z_n = x_neigh @ W (the 33 GFLOP term) entirely;
  the kernel becomes memory-bound on streaming h2 (131 MB).

Sharding: pure data-parallel over batch (128 batches/core x 8 cores).
Per-core dataflow: h2/h1/h0 streamed in bf16 in BOTH row-major (rows on
partitions; for the attention-weighted aggregation matmul, contracting
neighbors) and feature-major (features on partitions; for the e_n/e_s
projections, contracting features). Softmax runs rows-on-partitions in
125-row tiles (5 groups of 25 neighbors); group-broadcasts (e_s, 1/sum)
are done with tiny DRAM-bounce DMAs with replicated read APs.
"""

import sys

sys.path.insert(0, "/opt/trn_rl_repo")

from contextlib import ExitStack

import ml_dtypes
import numpy as np

import concourse.bass as bass
import concourse.tile as tile
from concourse import bacc, mybir
import concourse.bass_utils as bass_utils

BF = mybir.dt.bfloat16
F32 = mybir.dt.float32
AF = mybir.ActivationFunctionType

NCORES = 8
B, FEAT, HID, OUT, H = 1024, 128, 512, 256, 4
BC = B // NCORES              # 128 batches per core
G1 = BC * 10                  # 1280 level-1 groups (h1 rows)
R2 = G1 * 25                  # 32000 h2 rows
TR = 125                      # stage-A tile rows (5 groups of 25)
TPS = 32                      # tiles per superiter
NSUP = R2 // (TR * TPS)       # 8 superiters
SUPR = TR * TPS               # 4000 rows per superiter
SUPG = SUPR // 25             # 160 groups per superiter
TB = 80                       # stage-B/C tile rows (8 groups of 10)
NTB = G1 // TB                # 16 tiles
LEAKY = 0.2


def build_program():
    nc = bacc.Bacc(
        "TRN2",
        target_bir_lowering=False,
        debug=False,
        enable_asserts=False,
        num_devices=NCORES,
    )

    def din(name, shape, dt):
        return nc.dram_tensor(name, shape, dt, kind="ExternalInput").ap()

    x2r = din("x2r", [NSUP * TR, TPS * FEAT], BF)  # h2 rows-major, host-pretiled
    x2t = din("x2t", [FEAT, R2], BF)         # h2 feature-major bf16
    x1r = din("x1r", [TB, NTB * FEAT], BF)   # host-pretiled
    x1t = din("x1t", [FEAT, G1], BF)
    x0t = din("x0t", [FEAT, BC], BF)
    w0s4 = din("w0s4", [FEAT, H], BF)        # W0 . a0_s combos
    w0n4 = din("w0n4", [FEAT, H], BF)
    w1s4 = din("w1s4", [128, 4, H], BF)      # [128, kchunk, H] of [512, H]
    w0s4f = din("w0s4f", [FEAT, H], F32)
    w1s4f = din("w1s4f", [128, 4, H], F32)
    w1n4 = din("w1n4", [128, 4, H], BF)
    w0f = din("w0f", [FEAT, HID], F32)       # W0 fp32
    w1f = din("w1f", [128, 4, HID], F32)     # W1 k-chunked
    wfc = din("wfc", [128, 4, OUT], F32)     # W_fc k-chunked
    e5 = din("e5", [TR, 5], F32)             # group indicator (125 rows, 5 groups)
    e5x = din("e5x", [TR, TR], F32)          # block-constant expander (sums -> rows)
    e10x = din("e10x", [TB, TB], F32)
    e5at = din("e5at", [5, TR], BF)          # group->row expander (lhsT form)
    e10at = din("e10at", [8, TB], BF)
    e10 = din("e10", [TB, 8], F32)           # group indicator (80 rows, 8 groups)
    i128b = din("i128b", [128, 128], BF)     # identity bf16 (PE transpose)
    i128f = din("i128f", [128, 128], F32)    # identity fp32 (out transpose)
    out_d = nc.dram_tensor("out", [BC, OUT], F32, kind="ExternalOutput").ap()

    with tile.TileContext(nc) as tc, ExitStack() as ctx:
        const = ctx.enter_context(tc.tile_pool(name="const", bufs=1))
        perst = ctx.enter_context(tc.tile_pool(name="perst", bufs=1))
        stream = ctx.enter_context(tc.tile_pool(name="stream", bufs=4))
        sm = ctx.enter_context(tc.tile_pool(name="sm", bufs=2))
        smb = ctx.enter_context(tc.tile_pool(name="smb", bufs=2))
        ps = ctx.enter_context(tc.tile_pool(name="ps", bufs=2, space="PSUM"))
        dram = ctx.enter_context(tc.tile_pool(name="dram", bufs=1, space="DRAM"))
        dram2 = ctx.enter_context(tc.tile_pool(name="dram2", bufs=2, space="DRAM"))

        def cload(ap_in, shape, dt, name):
            t = const.tile(shape, dt, name=name, tag=name)
            nc.sync.dma_start(t[:], ap_in)
            return t

        w0s4_s = cload(w0s4, [FEAT, H], BF, name="w0s4")
        w0n4_s = cload(w0n4, [FEAT, H], BF, name="w0n4")
        w1s4_s = cload(w1s4, [128, 4, H], BF, name="w1s4")
        w1n4_s = cload(w1n4, [128, 4, H], BF, name="w1n4")
        w0f_s = cload(w0f, [FEAT, HID], F32, name="w0f")
        w1f_s = cload(w1f, [128, 4, HID], F32, name="w1f")
        wfc_s = cload(wfc, [128, 4, OUT], F32, name="wfc")
        e5_s = cload(e5, [TR, 5], F32, name="e5")
        e5x_s = cload(e5x, [TR, TR], F32, name="e5x")
        e10x_s = cload(e10x, [TB, TB], F32, name="e10x")
        e5at_s = cload(e5at, [5, TR], BF, name="e5at")
        e10at_s = cload(e10at, [8, TB], BF, name="e10at")
        w0s4f_s = cload(w0s4f, [FEAT, H], F32, name="w0s4f")
        w1s4f_s = cload(w1s4f, [128, 4, H], F32, name="w1s4f")
        e10_s = cload(e10, [TB, 8], F32, name="e10")
        i128b_s = cload(i128b, [128, 128], BF, name="i128b")
        i128f_s = cload(i128f, [128, 128], F32, name="i128f")
        x1t_s = cload(x1t, [FEAT, G1], BF, name="x1t")
        x0t_s = cload(x0t, [FEAT, BC], BF, name="x0t")
        x1r_s = cload(x1r.rearrange("p (i f) -> p i f", i=NTB), [TB, NTB, FEAT], BF, name="x1r")

        h1t_s = perst.tile([128, H, G1], BF)        # h1_new^T, feature-major bf16
        h1r_s = perst.tile([TB, NTB, H, 128], BF)   # h1_new row-major bf16

        mm = nc.tensor.matmul

        # ---------- STAGE B: layer0 on (h0 self, h1 neigh, E=10) ----------
        def level_softmax(en_ps_t, tag):
            """logits psum [TB,NTB,H] (e_n + e_s already accumulated) -> alpha_bd."""
            lr_ = sm.tile([TB, NTB, H], F32, tag=tag + "lr")
            nc.scalar.mul(lr_[:], en_ps_t[:], LEAKY)
            nc.vector.tensor_max(lr_[:], lr_[:], en_ps_t[:])
            p_ = sm.tile([TB, NTB, H], F32, tag=tag + "p")
            nc.scalar.activation(p_[:], lr_[:], AF.Exp)
            sm_ps_ = ps.tile([TB, NTB, H], F32, tag="sm", name=tag + "smps")
            mm(sm_ps_[:], e10x_s[:], p_[:], start=True, stop=True)
            rc_sb_ = sm.tile([TB, NTB, H], F32, tag=tag + "rc")
            nc.vector.reciprocal(rc_sb_[:], sm_ps_[:])
            al_ = sm.tile([TB, NTB, H], F32, tag=tag + "al")
            nc.vector.tensor_mul(al_[:], p_[:], rc_sb_[:])
            albd_ = sm.tile([TB, NTB, 8, H], BF, tag=tag + "albd")
            nc.vector.tensor_mul(
                albd_[:],
                al_[:].unsqueeze(2).broadcast_to((TB, NTB, 8, H)),
                e10_s[:].unsqueeze(1).unsqueeze(3).broadcast_to((TB, NTB, 8, H)),
            )
            return albd_

        esbam_ps = ps.tile([8, NTB, H], F32, tag="sm", name="esbam_ps")
        for i in range(NTB):
            mm(esbam_ps[:, i, :], x0t_s[:, 8 * i:8 * (i + 1)], w0s4_s[:],
               start=True, stop=True)
        esbam_sb = sm.tile([8, NTB, H], BF, tag="esbam_sb")
        nc.vector.tensor_copy(esbam_sb[:], esbam_ps[:])
        enb_ps = ps.tile([TB, NTB, H], F32, tag="en")
        for i in range(NTB):
            mm(enb_ps[:, i, :], x1t_s[:, TB * i:TB * (i + 1)], w0n4_s[:],
               start=True, stop=True)
        esbx_ps = ps.tile([TB, NTB, H], F32, tag="sm", name="esbx_ps")
        mm(esbx_ps[:], e10at_s[:], esbam_sb[:], start=True, stop=True)
        esbx_sb = sm.tile([TB, NTB, H], F32, tag="esbx_sb")
        nc.scalar.copy(esbx_sb[:], esbx_ps[:])
        lgb = sm.tile([TB, NTB, H], F32, tag="lgb")
        nc.vector.tensor_add(lgb[:], enb_ps[:], esbx_sb[:])
        albd_b = level_softmax(lgb, "b")

        xb_ps = ps.tile([128, NTB, 8, H], F32, tag="agg")
        for i in range(NTB):
            mm(xb_ps[:, i, :, :], x1r_s[:, i, :], albd_b[:, i, :, :],
               start=True, stop=True)
        xb_sb = smb.tile([128, NTB, 8, H], F32, tag="xbsb")
        nc.vector.tensor_copy(xb_sb[:], xb_ps[:])

        hb_ps = ps.tile([128, H, BC], F32, tag="hn")
        for h in range(H):
            mm(hb_ps[:, h, :], w0f_s[:, 128 * h:128 * (h + 1)], xb_sb[:, :, :, h],
               start=True, stop=True)
        h0t_bf = perst.tile([128, H, BC], BF)
        nc.vector.tensor_copy(h0t_bf[:], hb_ps[:])

        # ---------- STAGE A: layer0 on (h1 self, h2 neigh, E=25) ----------
        # e_s values for all superiters upfront: esam_all[5, s, t, h]
        esam_all = perst.tile([5, NSUP, TPS, H], BF)
        for half in range(2):
            esam_ps = ps.tile([5, NSUP // 2, TPS, H], F32, tag="sm", name="esam_ps")
            for s_ in range(NSUP // 2):
                s2 = NSUP // 2 * half + s_
                for t in range(TPS):
                    mm(esam_ps[:, s_, t, :],
                       x1t_s[:, SUPG * s2 + 5 * t:SUPG * s2 + 5 * t + 5],
                       w0s4_s[:], start=True, stop=True)
            nc.vector.tensor_copy(
                esam_all[:, NSUP // 2 * half:NSUP // 2 * (half + 1), :, :],
                esam_ps[:])

        # stage-C self attention values upfront (needs h0_new from stage B)
        escam_ps = ps.tile([8, NTB, H], F32, tag="sm", name="escam_ps")
        for i in range(NTB):
            for k in range(4):
                mm(escam_ps[:, i, :], h0t_bf[:, k, 8 * i:8 * (i + 1)],
                   w1s4_s[:, k, :], start=(k == 0), stop=(k == 3))
        escam_sb = sm.tile([8, NTB, H], BF, tag="escam_sb")
        nc.vector.tensor_copy(escam_sb[:], escam_ps[:])

        x2r_v = x2r.rearrange("(s p) (t f) -> s p t f", s=NSUP, t=TPS)
        xc_sb = smb.tile([128, 4, NTB, 8, H], F32, tag="xcsb")
        for s in range(NSUP):
            x2r_t = stream.tile([TR, TPS, FEAT], BF, tag="x2r")
            nc.sync.dma_start(x2r_t[:, :TPS // 2, :], x2r_v[s][:, :TPS // 2, :])
            nc.sync.dma_start(x2r_t[:, TPS // 2:, :], x2r_v[s][:, TPS // 2:, :])
            x2t_t = stream.tile([FEAT, SUPR], BF, tag="x2t")
            half = SUPR // 2
            nc.scalar.dma_start(x2t_t[:, :half],
                                x2t[:, SUPR * s:SUPR * s + half])
            nc.scalar.dma_start(x2t_t[:, half:],
                                x2t[:, SUPR * s + half:SUPR * (s + 1)])

            # e_n per tile, then expander accumulates e_s onto the same psum
            en_ps = ps.tile([TR, TPS, H], F32, tag="en")
            for t in range(TPS):
                mm(en_ps[:, t, :], x2t_t[:, TR * t:TR * (t + 1)], w0n4_s[:],
                   start=True, stop=True)
            esx_ps = ps.tile([TR, TPS, H], F32, tag="sm", name="esx_ps")
            mm(esx_ps[:], e5at_s[:], esam_all[:, s, :, :], start=True, stop=True)
            esx_sb = sm.tile([TR, TPS, H], F32, tag="esx_sb")
            nc.scalar.copy(esx_sb[:], esx_ps[:])
            lg = sm.tile([TR, TPS, H], F32, tag="lg")
            nc.vector.tensor_add(lg[:], en_ps[:], esx_sb[:])

            # softmax (rows-on-partitions, groups of 25)
            lr = sm.tile([TR, TPS, H], F32, tag="lr")
            nc.vector.tensor_scalar_mul(lr[:], lg[:], LEAKY)
            nc.vector.tensor_max(lr[:], lr[:], lg[:])
            p = sm.tile([TR, TPS, H], F32, tag="p")
            nc.scalar.activation(p[:], lr[:], AF.Exp)
            sm_ps = ps.tile([TR, TPS, H], F32, tag="sm")
            mm(sm_ps[:], e5x_s[:], p[:], start=True, stop=True)
            rc_sb = sm.tile([TR, TPS, H], F32, tag="rcsb")
            nc.vector.reciprocal(rc_sb[:], sm_ps[:])
            al = sm.tile([TR, TPS, H], F32, tag="al")
            nc.vector.tensor_mul(al[:], p[:], rc_sb[:])
            albd = sm.tile([TR, TPS, 5, H], BF, tag="albd")
            nc.vector.tensor_mul(
                albd[:],
                al[:].unsqueeze(2).broadcast_to((TR, TPS, 5, H)),
                e5_s[:].unsqueeze(1).unsqueeze(3).broadcast_to((TR, TPS, 5, H)),
            )

            # aggregation: x_agg^T[f, (tile, group, head)], contraction over rows
            xa_sb = smb.tile([128, TPS, 5, H], F32, tag="xasb")
            xa_flat = xa_sb[:].rearrange("p t g h -> p (t g h)")
            for j in range(2):
                xa_ps = ps.tile([128, TPS // 2, 20], F32, tag="agg", name=f"xaps{j}")
                for t2 in range(16):
                    t = 16 * j + t2
                    mm(xa_ps[:, t2, :], x2r_t[:, t, :], albd[:, t, :, :],
                       start=True, stop=True)
                nc.vector.tensor_copy(xa_flat[:, 320 * j:320 * (j + 1)],
                                      xa_ps[:].rearrange("p t x -> p (t x)"))

            # h1_new^T = W0_h^T @ x_agg_h : [128 d, 160 groups] per head
            for j in range(2):
                hn_ps = ps.tile([128, 2, SUPG], F32, tag="hn", name=f"hnps{j}")
                for h2_ in range(2):
                    h = 2 * j + h2_
                    mm(hn_ps[:, h2_, :], w0f_s[:, 128 * h:128 * (h + 1)],
                       xa_sb[:, :, :, h], start=True, stop=True)
                nc.vector.tensor_copy(
                    h1t_s[:, 2 * j:2 * (j + 1), SUPG * s:SUPG * (s + 1)], hn_ps[:])

            # transpose this superiter's h1_new slice to row-major (2 tiles)
            for i2 in range(2):
                i = 2 * s + i2
                tr_ps = ps.tile([TB, H, 128], F32, tag="hn", name="tr_ps")
                for h in range(H):
                    mm(tr_ps[:, h, :], h1t_s[:, h, TB * i:TB * (i + 1)],
                       i128b_s[:], start=True, stop=True)
                nc.scalar.copy(h1r_s[:, i, :, :], tr_ps[:])

            # ---- stage C (layer 1) for this superiter's two tiles ----
            encp = ps.tile([TB, 2, H], F32, tag="en", name="encp")
            for i2 in range(2):
                i = 2 * s + i2
                for k in range(4):
                    mm(encp[:, i2, :], h1t_s[:, k, TB * i:TB * (i + 1)],
                       w1n4_s[:, k, :], start=(k == 0), stop=(k == 3))
            escx_ps = ps.tile([TB, 2, H], F32, tag="sm", name="escx_ps")
            mm(escx_ps[:], e10at_s[:], escam_sb[:, 2 * s:2 * s + 2, :],
               start=True, stop=True)
            escx_sb = sm.tile([TB, 2, H], F32, tag="escx_sb")
            nc.scalar.copy(escx_sb[:], escx_ps[:])
            lgc = sm.tile([TB, 2, H], F32, tag="lgc")
            nc.vector.tensor_add(lgc[:], encp[:], escx_sb[:])
            lrc = sm.tile([TB, 2, H], F32, tag="lrc")
            nc.vector.tensor_scalar_mul(lrc[:], lgc[:], LEAKY)
            nc.vector.tensor_max(lrc[:], lrc[:], lgc[:])
            pc = sm.tile([TB, 2, H], F32, tag="pc")
            nc.scalar.activation(pc[:], lrc[:], AF.Exp)
            smc_ps = ps.tile([TB, 2, H], F32, tag="sm", name="smc_ps")
            mm(smc_ps[:], e10x_s[:], pc[:], start=True, stop=True)
            rcc = sm.tile([TB, 2, H], F32, tag="rcc")
            nc.vector.reciprocal(rcc[:], smc_ps[:])
            alc = sm.tile([TB, 2, H], F32, tag="alc")
            nc.vector.tensor_mul(alc[:], pc[:], rcc[:])
            albdc = sm.tile([TB, 2, 8, H], BF, tag="albdc")
            nc.vector.tensor_mul(
                albdc[:],
                alc[:].unsqueeze(2).broadcast_to((TB, 2, 8, H)),
                e10_s[:].unsqueeze(1).unsqueeze(3).broadcast_to((TB, 2, 8, H)),
            )
            xc_ps = ps.tile([128, 2, 4, 8, H], F32, tag="agg", name="xc_ps")
            for i2 in range(2):
                i = 2 * s + i2
                for k in range(4):
                    mm(xc_ps[:, i2, k, :, :], h1r_s[:, i, k, :],
                       albdc[:, i2, :, :], start=True, stop=True)
            nc.vector.tensor_copy(
                xc_sb[:, :, 2 * s:2 * s + 2, :, :].transpose([0, 2, 1, 3, 4]),
                xc_ps[:])

        # ---------- stage C epilogue: h0_fin = x_aggC @ W1 heads ----------
        hf_ps = ps.tile([128, H, BC], F32, tag="hn")
        for h in range(H):
            for k in range(4):
                mm(hf_ps[:, h, :], w1f_s[:, k, 128 * h:128 * (h + 1)],
                   xc_sb[:, k, :, :, h], start=(k == 0), stop=(k == 3))
        hf_sb = smb.tile([128, H, BC], F32, tag="hfsb")
        nc.vector.tensor_copy(hf_sb[:], hf_ps[:])

        # ---------- FC + output transpose ----------
        of_ps = ps.tile([128, 2, BC], F32, tag="agg")
        for m in range(2):
            for k in range(4):
                mm(of_ps[:, m, :], wfc_s[:, k, 128 * m:128 * (m + 1)], hf_sb[:, k, :],
                   start=(k == 0), stop=(k == 3))
        ot_sb = smb.tile([128, 2, BC], F32, tag="otsb")
        nc.vector.tensor_copy(ot_sb[:], of_ps[:])
        or_ps = ps.tile([BC, 2, 128], F32, tag="agg")
        for m in range(2):
            mm(or_ps[:, m, :], ot_sb[:, m, :], i128f_s[:], start=True, stop=True)
        or_sb = smb.tile([BC, 2, 128], F32, tag="orsb")
        nc.vector.tensor_copy(or_sb[:], or_ps[:])
        nc.sync.dma_start(out_d.rearrange("b (m o) -> b m o", m=2), or_sb[:])

    nc.compile()
    return nc


def _host_prep(h0, h1, h2, W0, a0_s, a0_n, W1, a1_s, a1_n, W_fc):
    bf16 = ml_dtypes.bfloat16
    f32 = np.float32

    def combo(W, a):  # [F, H*D], [H, D] -> [F, H]
        F_ = W.shape[0]
        return np.einsum("fhd,hd->fh", W.reshape(F_, H, 128), a).astype(f32)

    w0s = combo(W0, a0_s)
    w0n = combo(W0, a0_n)
    w1s = combo(W1, a1_s).reshape(4, 128, H).transpose(1, 0, 2)   # [128, 4, H]
    w1n = combo(W1, a1_n).reshape(4, 128, H).transpose(1, 0, 2)
    shared = {
        "w0s4": np.ascontiguousarray(w0s.astype(bf16)),
        "w0n4": np.ascontiguousarray(w0n.astype(bf16)),
        "w1s4": np.ascontiguousarray(w1s.astype(bf16)),
        "w1n4": np.ascontiguousarray(w1n.astype(bf16)),
        "w0f": np.ascontiguousarray(W0.astype(f32)),
        "w1f": np.ascontiguousarray(
            W1.reshape(4, 128, HID).transpose(1, 0, 2).astype(f32)),
        "wfc": np.ascontiguousarray(
            W_fc.reshape(4, 128, OUT).transpose(1, 0, 2).astype(f32)),
        "e5": (np.arange(TR)[:, None] // 25 == np.arange(5)[None, :]).astype(f32),
        "e10": (np.arange(TB)[:, None] // 10 == np.arange(8)[None, :]).astype(f32),
        "e5x": (np.arange(TR)[:, None] // 25 == np.arange(TR)[None, :] // 25).astype(f32),
        "e5at": (np.arange(5)[:, None] == np.arange(TR)[None, :] // 25).astype(bf16),
        "e10at": (np.arange(8)[:, None] == np.arange(TB)[None, :] // 10).astype(bf16),
        "w0s4f": w0s.astype(f32),
        "w1s4f": w1s.astype(f32),
        "e10x": (np.arange(TB)[:, None] // 10 == np.arange(TB)[None, :] // 10).astype(f32),
        "i128b": np.eye(128, dtype=bf16),
        "i128f": np.eye(128, dtype=f32),
    }
    in_maps = []
    for c in range(NCORES):
        sl = slice(c * BC, (c + 1) * BC)
        h2c = h2[sl].reshape(R2, FEAT).astype(f32)
        h1c = h1[sl].reshape(G1, FEAT).astype(f32)
        h0c = h0[sl].astype(f32)
        m = dict(shared)
        m["x2r"] = np.ascontiguousarray(
            h2c.astype(bf16).reshape(NSUP, TPS, TR, FEAT)
            .transpose(0, 2, 1, 3).reshape(NSUP * TR, TPS * FEAT))
        m["x2t"] = np.ascontiguousarray(h2c.T.astype(bf16))
        m["x1r"] = np.ascontiguousarray(
            h1c.astype(bf16).reshape(NTB, TB, FEAT)
            .transpose(1, 0, 2).reshape(TB, NTB * FEAT))
        m["x1t"] = np.ascontiguousarray(h1c.T.astype(bf16))
        m["x0t"] = np.ascontiguousarray(h0c.T.astype(bf16))
        in_maps.append(m)
    return in_maps


_PROGRAM = None


def kernel(**inputs):
    global _PROGRAM
    if _PROGRAM is None:
        _PROGRAM = build_program()
    in_maps = _host_prep(**{k: np.asarray(v) for k, v in inputs.items()})
    res = bass_utils.run_bass_kernel_spmd(
        _PROGRAM, in_maps, core_ids=list(range(NCORES)))
    return np.concatenate([r["out"] for r in res.results], axis=0)


if __name__ == "__main__":
    build_program()
    print("program built + compiled OK")



# revision 27
# speedup vs baseline: 1.6460x; 1.6460x over previous
"""Trainium2 Bass kernel for nn_GAT_7851200217746 (hierarchical GAT message passing).

Algorithm (aggregate-first GAT restructuring):
  For each GAT layer application on (x_self [G,F], x_neigh [G,E,F], W, a_s, a_n):
    w_s[f,h] = sum_d W[f,h*D+d] a_s[h,d];  w_n likewise
    e_s = x_self @ w_s;  e_n = x_neigh @ w_n
    alpha = softmax_E(leaky_relu(e_s + e_n))
    x_agg[g,h,:] = sum_e alpha[g,e,h] x_neigh[g,e,:]    (aggregate in INPUT space)
    out[g, h*D:(h+1)*D] = x_agg[g,h,:] @ W[:, h*D:(h+1)*D]

Perf structure (v2):
  - x2 streamed twice: row-major bf16 (aggregation contraction) and
    feature-major fp8e4m3 (e_n logits only; validated 0.7% rel err).
  - All matmuls bf16/fp8 (no fp32 on the PE), stationaries padded to 128
    columns so FWL fires (2x bf16 / 4x fp8 weight loads).
  - e_s terms computed batched (one mm per 80-128 groups) and broadcast
    to neighbor rows via indicator-product expander matmuls that
    accumulate directly into the logits PSUM (expander writes first with
    start=True, e_n matmuls accumulate after).
  - leaky_relu+exp on the Scalar (ACT) engine straight from PSUM.
  - One packed const DMA (gpsimd queue) + x2r on sync + x2t on scalar
    queues; all tiles 128-partition padded with zero-padding chosen so
    junk rows produce exactly 0 in albd (no NaN can propagate).

Sharding: pure data-parallel over batch (128 batches/core x 8 cores).
"""

import sys

sys.path.insert(0, "/opt/trn_rl_repo")

from contextlib import ExitStack

import ml_dtypes
import numpy as np

import concourse.bass as bass
import concourse.tile as tile
from concourse import bacc, mybir
import concourse.bass_utils as bass_utils

BF = mybir.dt.bfloat16
F32 = mybir.dt.float32
FP8 = mybir.dt.float8e4
AF = mybir.ActivationFunctionType

NCORES = 8
B, FEAT, HID, OUT, H = 1024, 128, 512, 256, 4
BC = B // NCORES              # 128 batches per core
G1 = BC * 10                  # 1280 level-1 groups (h1 rows)
R2 = G1 * 25                  # 32000 h2 rows
TR = 125                      # stage-A tile rows (5 groups of 25)
TPS = 32                      # tiles per superiter
NSUP = R2 // (TR * TPS)       # 8 superiters
SUPR = TR * TPS               # 4000 rows per superiter
SUPG = SUPR // 25             # 160 groups per superiter
X2TP = 4032                   # x2t cols per superiter incl zero pad
G1P = 1344                    # x1t padded cols (>= 1200+128)
TB = 80                       # stage-B/C tile rows (8 groups of 10)
NTB = G1 // TB                # 16 tiles
LEAKY = 0.2

# packed bf16 const/per-core "smalls" layout: name -> n_cols
SMALLS = [
    ("x1tp", G1P),            # h1^T feature-major, padded with zeros
    ("x0t", BC),              # h0^T feature-major
    ("x1r", NTB * FEAT),      # h1 row-major tiles [80 rows used, pad 0]
    ("w0s4", H), ("w0n4", H),
    ("w1s4", 4 * H), ("w1n4", 4 * H),     # [128, 4, H] k-chunked combos
    ("w0b", HID),                          # W0 bf16
    ("w1b", 4 * HID),                      # [128, 4, 512]
    ("wfcb", 4 * OUT),                     # [128, 4, 256]
    ("e5p", 5),               # row->group indicator, rows>=125 zero
    ("e5xp", 128),            # group-sum expander + identity pad
    ("L80p", 128),            # esam expander [g' mod 5 == r div 25]
    ("LBp", 128),             # stage-B/C expander [b mod 8 == r div 10]
    ("ind16", 16),            # [g' div 5 == t'] (rows>=80 zero)
    ("indB16", 16),           # [b div 8 == i]
    ("i128b", 128),           # identity
    ("e10p", 8),              # stage-B/C row->group indicator (rows>=80 zero)
    ("e10xp", 128),           # stage-B/C group-sum expander + identity pad
]
SOFF = {}
_off = 0
for _n, _c in SMALLS:
    SOFF[_n] = _off
    _off += _c
SCOLS = _off


def build_program(debug_out=False):
    nc = bacc.Bacc(
        "TRN2",
        target_bir_lowering=False,
        debug=False,
        enable_asserts=False,
        num_devices=NCORES,
    )

    x2r = nc.dram_tensor("x2r", [NSUP * TR, TPS * FEAT], BF,
                         kind="ExternalInput").ap()
    x2t8 = nc.dram_tensor("x2t8", [FEAT, NSUP * X2TP], FP8,
                          kind="ExternalInput").ap()
    smalls = nc.dram_tensor("smalls", [128, SCOLS], BF,
                            kind="ExternalInput").ap()
    out_d = nc.dram_tensor("out", [BC, OUT], F32, kind="ExternalOutput").ap()

    with tile.TileContext(nc) as tc, ExitStack() as ctx:
        const = ctx.enter_context(tc.tile_pool(name="const", bufs=1))
        perst = ctx.enter_context(tc.tile_pool(name="perst", bufs=1))
        stream = ctx.enter_context(tc.tile_pool(name="stream", bufs=4))
        sm = ctx.enter_context(tc.tile_pool(name="sm", bufs=2))
        smb = ctx.enter_context(tc.tile_pool(name="smb", bufs=2))
        # PSUM budget is 8 bank-slots (2KB each). Layout:
        #  psA "en" x2   — logits; the softmax-sum mm reuses the same tile
        #  psC "cen"/"cagg" x1 — stage-C logits (+sum reuse) and agg
        #  psb "agg"/"hn" x2  — x_agg + h1_new/transpose/epilogue (2KB)
        psA = ctx.enter_context(tc.tile_pool(name="psA", bufs=2, space="PSUM"))
        psC = ctx.enter_context(tc.tile_pool(name="psC", bufs=1, space="PSUM"))
        psb = ctx.enter_context(tc.tile_pool(name="psb", bufs=2, space="PSUM"))

        sm_s = const.tile([128, SCOLS], BF, name="smalls")
        nc.gpsimd.dma_start(sm_s[:], smalls)

        def sv(name, split=None):
            """Slice view of the packed smalls tile."""
            c = dict(SMALLS)[name]
            v = sm_s[:, SOFF[name]:SOFF[name] + c]
            if split is not None:
                v = v.rearrange("p (a b) -> p a b", a=split)
            return v

        x1tp = sv("x1tp")
        x0t = sv("x0t")
        x1r = sv("x1r", NTB)
        w0s4 = sv("w0s4")
        w0n4 = sv("w0n4")
        w1s4 = sv("w1s4", 4)
        w1n4 = sv("w1n4", 4)
        w0b = sv("w0b")
        w1b = sv("w1b", 4)
        wfcb = sv("wfcb", 4)
        e5p = sv("e5p")
        e5xp = sv("e5xp")
        L80p = sv("L80p")
        LBp = sv("LBp")
        ind16 = sv("ind16")
        indB16 = sv("indB16")
        i128b = sv("i128b")
        e10p = sv("e10p")
        e10xp = sv("e10xp")

        h1t_s = perst.tile([128, H, G1P], BF)       # h1_new^T feature-major
        # zero h1t once: stage-C stationaries are padded to 128 columns and
        # read 48 columns ahead of what this superiter wrote — stale SBUF
        # there could be inf/NaN, and 0*NaN in the PE poisons the softmax.
        nc.vector.memset(h1t_s[:], 0.0)
        h1r_s = perst.tile([TB, NTB, H, 128], BF)   # h1_new row-major
        xc_sb = perst.tile([128, 4, NTB, 8, H], BF)  # stage-C agg (d-major)
        R_all = perst.tile([128, 16, 16, H], BF)    # stage-A e_s expander rhs
        RB = perst.tile([128, 16, H], BF)           # stage-B e_s expander rhs
        RC = perst.tile([128, 16, H], BF)           # stage-C e_s expander rhs
        h0t_bf = perst.tile([128, H, BC], BF)       # stage-B output^T

        mm = nc.tensor.matmul

        # ---------- upfront: stage-A e_s for all superiters ----------
        # es80 chunk c: groups [80c, 80c+80).  R_all[g',c,t',h] =
        # e_s[80c+g', h] * [g' div 5 == t']
        es80_sb = smb.tile([128, 16, H], BF, tag="es80")
        for half in range(2):
            es_ps = psb.tile([128, 8, H], F32, tag="agg", name=f"es_ps{half}")
            for c_ in range(8):
                c = 8 * half + c_
                mm(es_ps[:, c_, :], x1tp[:, 80 * c:80 * c + 128], w0s4,
                   start=True, stop=True, skip_group_check=True)
            nc.vector.tensor_copy(es80_sb[:, 8 * half:8 * (half + 1), :],
                                  es_ps[:])
        nc.vector.tensor_mul(
            R_all[:],
            es80_sb[:].unsqueeze(2).broadcast_to((128, 16, 16, H)),
            ind16.unsqueeze(1).unsqueeze(3).broadcast_to((128, 16, 16, H)),
        )

        # ---------- STAGE B: layer0 on (h0 self, h1 neigh, E=10) ----------
        # esB = h0 @ w0s (one mm), RB = esB * indB16
        esB_ps = psb.tile([128, H], F32, tag="agg", name="esB_ps")
        mm(esB_ps[:], x0t, w0s4, start=True, stop=True, skip_group_check=True)
        esB_sb = smb.tile([128, H], BF, tag="esB")
        nc.vector.tensor_copy(esB_sb[:], esB_ps[:])
        nc.vector.tensor_mul(
            RB[:],
            esB_sb[:].unsqueeze(1).broadcast_to((128, 16, H)),
            indB16.unsqueeze(2).broadcast_to((128, 16, H)),
        )

        enb_t = psA.tile([128, NTB, H], F32, tag="en", name="enb")
        mm(enb_t[:], LBp, RB[:], start=True, stop=False,
           skip_group_check=True)
        for i in range(NTB):
            mm(enb_t[:, i, :], x1tp[:, TB * i:TB * i + 128], w0n4,
               start=False, stop=True, skip_group_check=True)
        pB = sm.tile([128, NTB, H], BF, tag="pB")
        lrB = sm.tile([128, NTB, H], F32, tag="lrB")
        nc.vector.tensor_scalar_mul(lrB[:], enb_t[:], LEAKY)
        nc.vector.tensor_max(lrB[:], lrB[:], enb_t[:])
        nc.scalar.activation(pB[:], lrB[:], AF.Exp)
        # group-sum reuses the logits PSUM region (logits dead after exp)
        mm(enb_t[:], e10xp, pB[:], start=True, stop=True,
           skip_group_check=True)
        rcB = sm.tile([128, NTB, H], F32, tag="rcB")
        nc.vector.reciprocal(rcB[:], enb_t[:])
        alB = sm.tile([128, NTB, H], BF, tag="alB")
        nc.vector.tensor_mul(alB[:], pB[:], rcB[:])
        albdB = sm.tile([128, NTB, 8, H], BF, tag="albdB")
        nc.vector.tensor_mul(
            albdB[:],
            alB[:].unsqueeze(2).broadcast_to((128, NTB, 8, H)),
            e10p.unsqueeze(1).unsqueeze(3).broadcast_to((128, NTB, 8, H)),
        )
        xb_ps = psb.tile([128, NTB, 8, H], F32, tag="agg")
        for i in range(NTB):
            mm(xb_ps[:, i, :, :], x1r[:, i, :], albdB[:, i, :, :],
               start=True, stop=True, skip_group_check=True)
        xb_bf = smb.tile([128, NTB, 8, H], BF, tag="xbbf")
        nc.vector.tensor_copy(xb_bf[:], xb_ps[:])
        hb_ps = psb.tile([128, H, BC], F32, tag="hn")
        for h in range(H):
            mm(hb_ps[:, h, :], w0b[:, 128 * h:128 * (h + 1)],
               xb_bf[:, :, :, h],
               start=True, stop=True, skip_group_check=True)
        nc.vector.tensor_copy(h0t_bf[:], hb_ps[:])

        # ---------- stage-C e_s (needs h0_new) ----------
        esC_ps = psb.tile([128, H], F32, tag="agg", name="esC_ps")
        for k in range(4):
            mm(esC_ps[:], h0t_bf[:, k, :], w1s4[:, k, :],
               start=(k == 0), stop=(k == 3), skip_group_check=True)
        esC_sb = smb.tile([128, H], BF, tag="esC")
        nc.vector.tensor_copy(esC_sb[:], esC_ps[:])
        nc.vector.tensor_mul(
            RC[:],
            esC_sb[:].unsqueeze(1).broadcast_to((128, 16, H)),
            indB16.unsqueeze(2).broadcast_to((128, 16, H)),
        )

        # ---------- STAGE A superiters ----------
        x2r_v = x2r.rearrange("(s p) tf -> s p tf", s=NSUP)
        x2t_v = x2t8.rearrange("p (s c) -> s p c", s=NSUP)
        for s in range(NSUP):
            x2r_t = stream.tile([TR, TPS, FEAT], BF, tag="x2r")
            nc.sync.dma_start(
                x2r_t[:].rearrange("p a b -> p (a b)"), x2r_v[s])
            x2t_t = stream.tile([FEAT, X2TP], FP8, tag="x2t")
            nc.scalar.dma_start(x2t_t[:], x2t_v[s])

            # logits: expander (e_s) first, then e_n accumulates.
            # ONE start=True mm for the whole tile: start marks pending-zero
            # at 2KB bank granularity, so two start=True writes into the
            # same bank lose the first one's data.
            en_t = psA.tile([128, TPS, H], F32, tag="en", name="en")
            mm(en_t[:], L80p, R_all[:, 2 * s:2 * s + 2, :, :],
               start=True, stop=False, skip_group_check=True)
            for t in range(TPS):
                mm(en_t[:, t, :], x2t_t[:, 125 * t:125 * t + 128], w0n4,
                   start=False, stop=True, skip_group_check=True)

            lr = sm.tile([128, TPS, H], F32, tag="lr")
            nc.vector.tensor_scalar_mul(lr[:], en_t[:], LEAKY)
            nc.vector.tensor_max(lr[:], lr[:], en_t[:])
            p = sm.tile([128, TPS, H], BF, tag="p")
            nc.scalar.activation(p[:], lr[:], AF.Exp)
            # group-sum reuses the logits PSUM region
            mm(en_t[:], e5xp, p[:], start=True, stop=True,
               skip_group_check=True)
            rc = sm.tile([128, TPS, H], F32, tag="rc")
            nc.vector.reciprocal(rc[:], en_t[:])
            al = sm.tile([128, TPS, H], BF, tag="al")
            nc.vector.tensor_mul(al[:], p[:], rc[:])
            albd = sm.tile([128, TPS, 5, H], BF, tag="albd")
            nc.vector.tensor_mul(
                albd[:],
                al[:].unsqueeze(2).broadcast_to((128, TPS, 5, H)),
                e5p.unsqueeze(1).unsqueeze(3).broadcast_to((128, TPS, 5, H)),
            )

            if debug_out and s == 0:
                dbg_lr = nc.dram_tensor("dbg_lr", [128, TPS * H], F32,
                                        kind="ExternalOutput").ap()
                nc.sync.dma_start(
                    dbg_lr.rearrange("p (a b) -> p a b", a=TPS), lr[:])
                dbg_al = nc.dram_tensor("dbg_al", [128, TPS * 5 * H], BF,
                                        kind="ExternalOutput").ap()
                nc.sync.dma_start(
                    dbg_al.rearrange("p (a b c) -> p a b c", a=TPS, b=5),
                    albd[:])

            # aggregation: x_agg^T[f, (t, g, h)]
            xa_bf = smb.tile([128, TPS, 5, H], BF, tag="xabf")
            for j in range(2):
                xa_ps = psb.tile([128, TPS // 2, 20], F32, tag="agg",
                                 name=f"xa{j}")
                for t2 in range(16):
                    t = 16 * j + t2
                    mm(xa_ps[:, t2, :], x2r_t[:, t, :],
                       albd[:TR, t, :, :], start=True, stop=True,
                       skip_group_check=True)
                nc.scalar.copy(
                    xa_bf[:, 16 * j:16 * (j + 1), :, :].rearrange(
                        "p a b c -> p (a b c)"),
                    xa_ps[:].rearrange("p t x -> p (t x)"))

            # h1_new^T = W0_h^T @ x_agg_h
            for j in range(2):
                hn_ps = psb.tile([128, 2, SUPG], F32, tag="hn",
                                 name=f"hnps{j}")
                for h2_ in range(2):
                    h = 2 * j + h2_
                    mm(hn_ps[:, h2_, :], w0b[:, 128 * h:128 * (h + 1)],
                       xa_bf[:, :, :, h],
                       start=True, stop=True, skip_group_check=True)
                nc.vector.tensor_copy(
                    h1t_s[:, 2 * j:2 * (j + 1), SUPG * s:SUPG * (s + 1)],
                    hn_ps[:])

            # transpose h1_new slice to row-major (2 tiles of 80 groups)
            for i2 in range(2):
                i = 2 * s + i2
                tr_ps = psb.tile([128, H, 128], F32, tag="hn", name="tr_ps")
                for h in range(H):
                    mm(tr_ps[:, h, :], h1t_s[:, h, TB * i:TB * i + 128],
                       i128b, start=True, stop=True, skip_group_check=True)
                nc.scalar.copy(h1r_s[:, i, :, :], tr_ps[:TB, :, :])

            # ---- stage C (layer 1) for this superiter's two tiles ----
            encp = psC.tile([128, 2, H], F32, tag="cen", name="encp")
            mm(encp[:], LBp, RC[:, 2 * s:2 * s + 2, :], start=True,
               stop=False, skip_group_check=True)
            for i2 in range(2):
                i = 2 * s + i2
                for k in range(4):
                    mm(encp[:, i2, :], h1t_s[:, k, TB * i:TB * i + 128],
                       w1n4[:, k, :], start=False, stop=(k == 3),
                       skip_group_check=True)
            lrc = sm.tile([128, 2, H], F32, tag="lrc")
            nc.vector.tensor_scalar_mul(lrc[:], encp[:], LEAKY)
            nc.vector.tensor_max(lrc[:], lrc[:], encp[:])
            pc = sm.tile([128, 2, H], BF, tag="pc")
            nc.scalar.activation(pc[:], lrc[:], AF.Exp)
            # group-sum reuses the stage-C logits PSUM region
            mm(encp[:], e10xp, pc[:], start=True, stop=True,
               skip_group_check=True)
            rcc = sm.tile([128, 2, H], F32, tag="rcc")
            nc.vector.reciprocal(rcc[:], encp[:])
            alc = sm.tile([128, 2, H], BF, tag="alc")
            nc.vector.tensor_mul(alc[:], pc[:], rcc[:])
            albdc = sm.tile([128, 2, 8, H], BF, tag="albdc")
            nc.vector.tensor_mul(
                albdc[:],
                alc[:].unsqueeze(2).broadcast_to((128, 2, 8, H)),
                e10p.unsqueeze(1).unsqueeze(3).broadcast_to((128, 2, 8, H)),
            )
            xc_ps = psC.tile([128, 2, 4, 8, H], F32, tag="cagg", name="xc_ps")
            for i2 in range(2):
                i = 2 * s + i2
                for k in range(4):
                    mm(xc_ps[:, i2, k, :, :], h1r_s[:, i, k, :],
                       albdc[:TB, i2, :, :], start=True, stop=True,
                       skip_group_check=True)
            nc.vector.tensor_copy(
                xc_sb[:, :, 2 * s:2 * s + 2, :, :].transpose([0, 2, 1, 3, 4]),
                xc_ps[:])

        # ---------- stage C epilogue: h0_fin = x_aggC @ W1 heads ----------
        hf_ps = psb.tile([128, H, BC], F32, tag="hn")
        for h in range(H):
            for k in range(4):
                mm(hf_ps[:, h, :], w1b[:, k, 128 * h:128 * (h + 1)],
                   xc_sb[:, k, :, :, h],
                   start=(k == 0), stop=(k == 3), skip_group_check=True)
        hf_bf = smb.tile([128, H, BC], BF, tag="hfbf")
        nc.scalar.copy(hf_bf[:], hf_ps[:])

        # ---------- FC + output transpose ----------
        of_ps = psb.tile([128, 2, BC], F32, tag="agg")
        for m in range(2):
            for k in range(4):
                mm(of_ps[:, m, :], wfcb[:, k, 128 * m:128 * (m + 1)],
                   hf_bf[:, k, :], start=(k == 0), stop=(k == 3),
                   skip_group_check=True)
        ot_bf = smb.tile([128, 2, BC], BF, tag="otbf")
        nc.vector.tensor_copy(ot_bf[:], of_ps[:])
        or_ps = psb.tile([BC, 2, 128], F32, tag="hn", name="or_ps")
        for m in range(2):
            mm(or_ps[:, m, :], ot_bf[:, m, :], i128b, start=True, stop=True,
               skip_group_check=True)
        or_sb = smb.tile([BC, 2, 128], F32, tag="orsb")
        nc.vector.tensor_copy(or_sb[:], or_ps[:])
        nc.sync.dma_start(out_d.rearrange("b (m o) -> b m o", m=2), or_sb[:])

        if debug_out:
            dbg_h1t = nc.dram_tensor("dbg_h1t", [128, H * G1P], BF,
                                     kind="ExternalOutput").ap()
            dbg_h0t = nc.dram_tensor("dbg_h0t", [128, H * BC], BF,
                                     kind="ExternalOutput").ap()
            dbg_xc = nc.dram_tensor("dbg_xc", [128, 4 * NTB * 8 * H], BF,
                                    kind="ExternalOutput").ap()
            nc.sync.dma_start(
                dbg_h1t.rearrange("p (a b) -> p a b", a=H), h1t_s[:])
            nc.sync.dma_start(
                dbg_h0t.rearrange("p (a b) -> p a b", a=H), h0t_bf[:])
            nc.sync.dma_start(
                dbg_xc.rearrange("p (a b c d) -> p a b c d", a=4, b=NTB, c=8),
                xc_sb[:])

    nc.compile()
    return nc


def _host_prep(h0, h1, h2, W0, a0_s, a0_n, W1, a1_s, a1_n, W_fc):
    bf16 = ml_dtypes.bfloat16
    fp8 = ml_dtypes.float8_e4m3
    f32 = np.float32

    def combo(W, a):  # [F, H*D], [H, D] -> [F, H]
        F_ = W.shape[0]
        return np.einsum("fhd,hd->fh", W.reshape(F_, H, 128), a).astype(f32)

    w0s = combo(W0, a0_s)
    w0n = combo(W0, a0_n)
    w1s = combo(W1, a1_s).reshape(4, 128, H).transpose(1, 0, 2)   # [128,4,H]
    w1n = combo(W1, a1_n).reshape(4, 128, H).transpose(1, 0, 2)

    ar = np.arange
    sm_shared = {}

    def put(name, arr):
        a = np.zeros((128, dict(SMALLS)[name]), dtype=bf16)
        a[:arr.shape[0], :arr.shape[1]] = arr.astype(bf16)
        sm_shared[name] = a

    put("w0s4", w0s)
    put("w0n4", w0n)
    put("w1s4", w1s.reshape(128, 4 * H))
    put("w1n4", w1n.reshape(128, 4 * H))
    put("w0b", W0.astype(f32))
    put("w1b", W1.reshape(4, 128, HID).transpose(1, 0, 2).reshape(128, -1))
    put("wfcb", W_fc.reshape(4, 128, OUT).transpose(1, 0, 2).reshape(128, -1))
    e5p = (ar(128)[:, None] // 25 == ar(5)[None, :]) & (ar(128)[:, None] < 125)
    put("e5p", e5p.astype(f32))
    e5x = np.zeros((128, 128), dtype=f32)
    blk = (ar(125)[:, None] // 25 == ar(125)[None, :] // 25)
    e5x[:125, :125] = blk
    e5x[125:, :] = 0.0
    for m in range(125, 128):
        e5x[m, m] = 1.0
    put("e5xp", e5x)
    L80 = np.zeros((128, 128), dtype=f32)
    L80[:80, :125] = (ar(80)[:, None] % 5 == ar(125)[None, :] // 25)
    put("L80p", L80)
    LB = np.zeros((128, 128), dtype=f32)
    LB[:, :80] = (ar(128)[:, None] % 8 == ar(80)[None, :] // 10)
    put("LBp", LB)
    ind16 = np.zeros((128, 16), dtype=f32)
    ind16[:80] = (ar(80)[:, None] // 5 == ar(16)[None, :])
    put("ind16", ind16)
    put("indB16", (ar(128)[:, None] // 8 == ar(16)[None, :]).astype(f32))
    put("i128b", np.eye(128, dtype=f32))
    e10 = np.zeros((128, 8), dtype=f32)
    e10[:80] = (ar(80)[:, None] // 10 == ar(8)[None, :])
    put("e10p", e10)
    e10x = np.zeros((128, 128), dtype=f32)
    e10x[:80, :80] = (ar(80)[:, None] // 10 == ar(80)[None, :] // 10)
    for m in range(80, 128):
        e10x[m, m] = 1.0
    put("e10xp", e10x)

    in_maps = []
    for c in range(NCORES):
        sl = slice(c * BC, (c + 1) * BC)
        h2c = np.asarray(h2[sl], dtype=f32).reshape(R2, FEAT)
        h1c = np.asarray(h1[sl], dtype=f32).reshape(G1, FEAT)
        h0c = np.asarray(h0[sl], dtype=f32)
        m = dict(sm_shared)
        x1tp = np.zeros((128, G1P), dtype=bf16)
        x1tp[:, :G1] = h1c.T.astype(bf16)
        m["x1tp"] = x1tp
        m["x0t"] = np.zeros((128, BC), dtype=bf16)
        m["x0t"][:] = h0c.T.astype(bf16)
        x1r = np.zeros((128, NTB, FEAT), dtype=bf16)
        x1r[:TB] = h1c.astype(bf16).reshape(NTB, TB, FEAT).transpose(1, 0, 2)
        m["x1r"] = x1r.reshape(128, NTB * FEAT)
        # pack the smalls in layout order
        packed = np.concatenate([m.pop(n) for n, _ in SMALLS], axis=1)
        mm_ = {"smalls": np.ascontiguousarray(packed)}
        mm_["x2r"] = np.ascontiguousarray(
            h2c.astype(bf16).reshape(NSUP, TPS, TR, FEAT)
            .transpose(0, 2, 1, 3).reshape(NSUP * TR, TPS * FEAT))
        x2t = np.zeros((FEAT, NSUP, X2TP), dtype=fp8)
        x2t[:, :, :SUPR] = h2c.T.astype(fp8).reshape(FEAT, NSUP, SUPR)
        mm_["x2t8"] = np.ascontiguousarray(x2t.reshape(FEAT, NSUP * X2TP))
        in_maps.append(mm_)
    return in_maps


_PROGRAM = None


def kernel(**inputs):
    global _PROGRAM
    if _PROGRAM is None:
        _PROGRAM = build_program()
    in_maps = _host_prep(**{k: np.asarray(v) for k, v in inputs.items()})
    res = bass_utils.run_bass_kernel_spmd(
        _PROGRAM, in_maps, core_ids=list(range(NCORES)))
    return np.concatenate([r["out"] for r in res.results], axis=0)


if __name__ == "__main__":
    build_program()
    print("program built + compiled OK")
